# revision 21
# baseline (speedup 1.0000x reference)
"""Distributed 21-qubit Pauli-rotation statevector kernel for 8 TRN2 NeuronCores.

Single-dispatch design: the statevector (2^21 complex, two fp32 planes packed
in a [128, 4096] tile per core) is sharded once by a GF(2) parity-check map H
chosen so a maximal subset of the 32 rotation gates is core-local.  All 32
gates, plus the 8 measurement expectation values, run in ONE bass program:

  local gate:   t = AB * R                    (VectorE)
                psum = (c*I) @ AB + SignedPerm @ t[cols ^ flip]   (TensorE)
                AB' = copy(psum)              (ScalarE)
  cross gate:   partner shard fetched with a pairwise AllReduce(add) over
                replica pairs {a, a^d} followed by S = sum - AB (avoids any
                per-core positional logic), then same flip machinery with
                source S instead of AB.
  measurement:  grouped by cross-core offset d (one pairwise exchange per
                distinct d), per-partition partials reduced on device,
                summed on the host in float64.

All large inputs (statevector planes, per-gate coefficient rows, per-gate
128x128 diag/perm matrices) are cached device-resident keyed on input
content, so a steady-state call is one dispatch + a 32 KB fetch.
"""
import dataclasses
import numpy as np

NW = 21
DIM = 1 << NW
P = 128
NF = 2048
NCOL = 4096
NLOC = 18
N_CORES = 8
N_GATES = 32
N_MEAS = 8

# ---------------------------------------------------------------- GF(2) utils
def parity(x):
    return bin(x).count("1") & 1

def parity_vec(x):
    x = x.copy()
    for s in (16, 8, 4, 2, 1):
        x ^= x >> s
    return x & 1

def gf2_basis(vs):
    basis = []
    for v in vs:
        for b in basis:
            v = min(v, v ^ b)
        if v:
            basis.append(v)
            basis.sort(reverse=True)
    return basis

def gf2_reduce(basis, v):
    for b in basis:
        v = min(v, v ^ b)
    return v

def annihilator(flips, n=NW):
    B = gf2_basis(flips)
    B = sorted(B, reverse=True)
    for i in range(len(B)):
        p = B[i].bit_length() - 1
        for k in range(len(B)):
            if k != i and (B[k] >> p) & 1:
                B[k] ^= B[i]
    piv = [b.bit_length() - 1 for b in B]
    out = []
    for fb in [i for i in range(n) if i not in piv]:
        h = 1 << fb
        for b in B:
            if (b >> fb) & 1:
                h ^= 1 << (b.bit_length() - 1)
        assert all(parity(h & f) == 0 for f in flips)
        out.append(h)
    return out

def gf2_inv3(A):
    n = 3
    M = [[int(A[r][c]) for c in range(n)] + [1 if r == c else 0 for c in range(n)]
         for r in range(n)]
    for col in range(n):
        p = next(r for r in range(col, n) if M[r][col])
        M[col], M[p] = M[p], M[col]
        for r in range(n):
            if r != col and M[r][col]:
                M[r] = [a ^ b for a, b in zip(M[r], M[col])]
    return [[M[r][n + c] for c in range(n)] for r in range(n)]

class Phase:
    def __init__(self, name, flips_to_cover=None, H=None):
        self.name = name
        if H is None:
            ann = sorted(annihilator(flips_to_cover), key=lambda h: bin(h).count("1"))
            H = []
            for h in ann:
                if len(gf2_basis(H + [h])) == len(H) + 1:
                    H.append(h)
                if len(H) == 3:
                    break
        assert len(H) == 3
        self.H = list(H)
        piv = []
        M = list(H)
        for r in range(3):
            for b in range(NW - 1, -1, -1):
                if b not in piv and (M[r] >> b) & 1:
                    piv.append(b)
                    for r2 in range(3):
                        if r2 != r and (M[r2] >> b) & 1:
                            M[r2] ^= M[r]
                    break
        self.pivots = piv
        self.literals = [i for i in range(NW) if i not in piv]
        self.lit_pos = list(self.literals)
        A = [[(self.H[r] >> self.pivots[q]) & 1 for q in range(3)] for r in range(3)]
        self.Ainv = gf2_inv3(A)

    def core_of(self, j):
        return sum(parity(j & self.H[r]) << r for r in range(3))

    def global_of_vec(self, core, l):
        j = np.zeros_like(l)
        for k, pos in enumerate(self.lit_pos):
            j |= ((l >> k) & 1) << pos
        c = np.zeros_like(l)
        for r in range(3):
            c |= parity_vec(j & self.H[r]) << r
        rhs = (core ^ c).astype(j.dtype)
        for r in range(3):
            xr = np.zeros_like(l)
            for q in range(3):
                if self.Ainv[r][q]:
                    xr ^= (rhs >> q) & 1
            j |= xr << self.pivots[r]
        return j

def gate_local(phase, F, PM, ny, strict=True):
    if strict:
        assert all(parity(F & h) == 0 for h in phase.H), "flip not core-local"
    fl = 0
    for k, pos in enumerate(phase.lit_pos):
        fl |= ((F >> pos) & 1) << k
    u = [(PM >> phase.pivots[q]) & 1 for q in range(3)]
    w = [0, 0, 0]
    for r in range(3):
        acc = 0
        for q in range(3):
            acc ^= int(u[q]) & int(phase.Ainv[q][r])
        w[r] = int(acc)
    pm_local = 0
    for k, pos in enumerate(phase.lit_pos):
        b = (PM >> pos) & 1
        for r in range(3):
            b ^= w[r] & ((phase.H[r] >> pos) & 1)
        pm_local |= b << k
    core_sign = np.array([
        (-1.0) ** ((((c >> 0) & 1) * w[0]) ^ (((c >> 1) & 1) * w[1]) ^ (((c >> 2) & 1) * w[2]))
        for c in range(8)])
    return dict(mf=fl & 0x7FF, mp=fl >> 11, pmf=pm_local & 0x7FF, pmp=pm_local >> 11,
                core_sign=core_sign)

# ------------------------------------------------------- XOR access patterns
def _runs(mask, nbits):
    runs = []
    bit = nbits - 1
    while bit >= 0:
        v = (mask >> bit) & 1
        lo = bit
        while lo >= 0 and ((mask >> lo) & 1) == v:
            lo -= 1
        runs.append((v, lo + 1, bit))
        bit = lo
    return runs

def xor_dims(mask, nbits, stride=1):
    dims = []
    for v, lo, hi in _runs(mask, nbits):
        count = 1 << (hi - lo + 1)
        step = (1 << lo) * stride
        dims.append([-step if v else step, count])
    return dims

def split_inner(m, nbits):
    if m == 0:
        return [(0, 0, [[1, 1 << nbits]], [[1, 1 << nbits]], 1 << nbits)]
    for c in range(nbits, -1, -1):
        mc = m & ((1 << c) - 1)
        ok = None
        for b in (0,):
            hi_mask = mc >> b << b
            lo_mask = mc & ((1 << b) - 1)
            od = xor_dims(lo_mask, c) if lo_mask else [[1, 1 << c]]
            idd = xor_dims(hi_mask, c) if hi_mask else [[1, 1 << c]]
            if len(od) <= 3 and len(idd) <= 3:
                ok = (hi_mask, lo_mask, od, idd)
                break
        if ok is not None:
            hi_mask, lo_mask, od, idd = ok
            mhi_all = m >> c
            return [((hi << c) + lo_mask, ((hi ^ mhi_all) << c) + hi_mask, od, idd,
                     1 << c) for hi in range(1 << (nbits - c))]
    raise AssertionError(m)

def window_calls(mask12, wbits=9):
    """Per-512-window xor-gather calls: (out_off, in_off, out_dims, in_dims, cnt)."""
    win = 1 << wbits
    inner = split_inner(mask12 & (win - 1), wbits)
    mhi = mask12 >> wbits
    calls = []
    for wi in range(NCOL // win):
        for (oo, io, od, idd, cnt) in inner:
            calls.append((wi * win + oo, ((wi ^ mhi) * win) + io, od, idd, cnt))
    return calls

def ap_with(ap, offset_add, dims):
    part = list(ap.ap[0])
    return dataclasses.replace(ap, offset=ap.offset + offset_add,
                               ap=[part] + [list(d) for d in dims])

# ------------------------------------------------------------- host planning
def build_R(g, core, coeff_a, coeff_b):
    f = np.arange(NF, dtype=np.int64)
    sf = 1.0 - 2.0 * parity_vec(f & g['pmf'])
    K = g['core_sign'][core] * ((-1.0) ** parity(g['mf'] & g['pmf']))
    return np.concatenate([coeff_a * K * sf, coeff_b * K * sf]).astype(np.float32)

def gate_coeffs(ny, cth, sth):
    """(chi, coeff_srccomp0, coeff_srccomp1) for a rotation gate."""
    if ny % 2 == 1:
        wr = -sth if ny % 4 == 1 else sth
        return 0, wr, wr
    wi = -sth if ny % 4 == 0 else sth
    return 1, wi, -wi     # src comp0 feeds b-out (+wi), src comp1 feeds a-out (-wi)

def meas_coeffs(ny):
    if ny % 2 == 0:
        return 0, 1.0, 1.0
    return 1, -1.0, 1.0   # src comp0 feeds b-out (-1), src comp1 feeds a-out (+1)

def build_mats(g, cth, core):
    sp = 1.0 - 2.0 * parity_vec(np.arange(P, dtype=np.int64) & g['pmp'])
    perm = np.zeros((P, P), np.float32)
    pr = np.arange(P)
    perm[pr ^ g['mp'], pr] = sp.astype(np.float32)
    diag = (np.eye(P) * cth).astype(np.float32)
    return diag, perm

_SPLIT_LEN_CACHE = {}

def _ncalls9(m9):
    """Matmul sub-calls per 512-col window for a given low-9-bit flip mask."""
    r = _SPLIT_LEN_CACHE.get(m9)
    if r is None:
        r = len(split_inner(m9, 9))
        _SPLIT_LEN_CACHE[m9] = r
    return r

def _optimize_bit_order(ph, flips):
    """Reorder the 18 local coordinate bits to minimize TensorE instruction
    count: bits 11-17 land in partitions (flips there are free via the perm
    matmul) and bits 9-10 only permute windows, so only the low 9 column bits
    fragment the xor-gather into extra matmul calls."""
    import random
    fls = [sum(((F >> pos) & 1) << k for k, pos in enumerate(ph.lit_pos))
           for F in flips]

    def cost(perm):
        tot = 0
        for fl in fls:
            m9 = 0
            for k in range(9):
                m9 |= ((fl >> perm[k]) & 1) << k
            tot += _ncalls9(m9)
        return tot

    rng = random.Random(7)
    cur = best = list(range(18))
    cc = bc = cost(cur)
    for _ in range(25000):
        i, j = rng.randrange(18), rng.randrange(18)
        cand = list(cur)
        cand[i], cand[j] = cand[j], cand[i]
        x = cost(cand)
        if x <= cc or rng.random() < 0.02:
            cur, cc = cand, x
            if x < bc:
                best, bc = list(cand), x
    ph.lit_pos = [ph.lit_pos[k] for k in best]
    return ph

def _pairs_for_d(d):
    return [[a, a ^ d] for a in range(8) if a < (a ^ d)]

# the runtime rejects pairwise replica groups for d=6 (0<->6,1<->7,2<->4,3<->5)
# and d=7; reach those partners with two chained supported exchanges instead
_HOPS = {1: [1], 2: [2], 3: [3], 4: [4], 5: [5], 6: [2, 4], 7: [3, 4]}

def _exchange_cost(d):
    return 0 if d == 0 else len(_HOPS[d])

def _plan_sharding(gf, mf):
    """Pick local gate set (max greedy coverage, rank<=18) and the Phase."""
    import random
    rng = random.Random(12345)
    best_S, best_basis = None, None
    order0 = list(range(N_GATES))
    for trial in range(2000):
        order = list(order0)
        if trial:
            rng.shuffle(order)
        basis, S = [], []
        for i in order:
            red = gf2_reduce(basis, gf[i])
            if red == 0:
                S.append(i)
            elif len(basis) < NLOC:
                basis.append(red)
                basis.sort(reverse=True)
                S.append(i)
        if best_S is None or len(S) > len(best_S):
            best_S, best_basis = sorted(S), list(basis)
            if len(best_S) == N_GATES:
                break
    # extend span with measurement flips where possible (localizes them)
    basis = list(best_basis)
    locset = [gf[i] for i in best_S]
    for m in range(N_MEAS):
        red = gf2_reduce(basis, mf[m])
        if red != 0 and len(basis) < NLOC:
            basis.append(red)
            basis.sort(reverse=True)
        if gf2_reduce(basis, mf[m]) == 0:
            locset.append(mf[m])
    ph0 = Phase('S0', flips_to_cover=locset)
    # re-pick the annihilator basis (core labeling) to minimize exchange hops:
    # gates pay per occurrence, measurements once per distinct nonzero d
    span = [ph0.H[0], ph0.H[1], ph0.H[2],
            ph0.H[0] ^ ph0.H[1], ph0.H[0] ^ ph0.H[2], ph0.H[1] ^ ph0.H[2],
            ph0.H[0] ^ ph0.H[1] ^ ph0.H[2]]
    best = None
    for h0 in span:
        for h1 in span:
            for h2 in span:
                if len(gf2_basis([h0, h1, h2])) != 3:
                    continue
                H = [h0, h1, h2]
                def d_of(F):
                    return sum(parity(F & H[r]) << r for r in range(3))
                cost = sum(_exchange_cost(d_of(gfl)) for gfl in gf)
                cost += sum(_exchange_cost(d)
                            for d in {d_of(mfl) for mfl in mf})
                if best is None or cost < best[0]:
                    best = (cost, H)
    ph = Phase('S', H=best[1])
    return _optimize_bit_order(ph, list(gf) + list(mf))

# ------------------------------------------------------------- bass builder
def _build_nc(gates, meas, meas_d_order):
    """One program: 32 gates (local or pairwise-exchange) + 8 measurements.

    gates: list of dicts mf, mp, chi, d (per gate).
    meas:  list of dicts mf, mp, chi, d (per measurement).
    meas_d_order: distinct d values in processing order (0 first if present).
    """
    import concourse.bass as bass
    import concourse.bacc as bacc
    import concourse.tile as tile
    import concourse.mybir as mybir
    DT = mybir.dt.float32
    n_g = len(gates)
    n_m = len(meas)
    n_mats = 2 * n_g + n_m
    nc = bacc.Bacc(None, target_bir_lowering=False)
    ab_in = nc.dram_tensor("ab_in", [P, NCOL], DT, kind="ExternalInput")
    r_rows = nc.dram_tensor("r_rows", [n_g + n_m, NCOL], DT, kind="ExternalInput")
    mats = nc.dram_tensor("mats", [n_mats, P, P], DT, kind="ExternalInput")
    cth_in = nc.dram_tensor("cth", [P, n_g], DT, kind="ExternalInput")
    acc_out = nc.dram_tensor("acc_out", [P, n_m], DT, kind="ExternalOutput")

    with tile.TileContext(nc) as tc:
        with tc.tile_pool(name="sb", bufs=1) as pool, \
             tc.tile_pool(name="rpool", bufs=3) as rpool, \
             tc.tile_pool(name="dram", bufs=4, space="DRAM") as dram, \
             tc.tile_pool(name="ps", bufs=1, space="PSUM") as psp:
            AB = pool.tile([P, NCOL], DT, tag="AB")
            AB2 = pool.tile([P, NCOL], DT, tag="AB2")
            t = pool.tile([P, NCOL], DT, tag="t")
            S = pool.tile([P, NCOL], DT, tag="S")
            t2 = pool.tile([P, NCOL], DT, tag="t2")
            M = pool.tile([P, n_mats * P], DT, tag="M")
            Cth = pool.tile([P, n_g], DT, tag="Cth")
            accs = pool.tile([P, n_m], DT, tag="accs")
            ps0 = psp.tile([P, 2048], DT, tag="ps0")
            ps1 = psp.tile([P, 2048], DT, tag="ps1")

            nc.sync.dma_start(AB[:], ab_in[:, :])
            matsap = dataclasses.replace(
                M[:], ap=[list(M[:].ap[0]), [P, n_mats], [1, P]])
            nc.sync.dma_start(matsap, dataclasses.replace(
                mats[:, :, :], ap=[[P, P], [P * P, n_mats], [1, P]]))
            nc.sync.dma_start(Cth[:], cth_in[:, :])

            def load_R(idx):
                Rt = rpool.tile([P, NCOL], DT, tag="R")
                nc.sync.dma_start(Rt[:], r_rows[idx:idx + 1, :].to_broadcast((P, NCOL)))
                return Rt

            # single shared bounce pair: reusing the same DRAM buffers for every
            # exchange serializes the collectives (overlapping in-flight
            # collectives with different replica groups can wedge the device)
            cc_in = dram.tile([P, NCOL], DT, tag="cc_in")
            cc_out = dram.tile([P, NCOL], DT, tag="cc_out")

            def fetch_partner(cur, d):
                """S <- shard of core (self ^ d), chaining pairwise AllReduces."""
                srcT = cur
                for h in _HOPS[d]:
                    nc.gpsimd.dma_start(cc_in[:], srcT[:])
                    nc.gpsimd.collective_compute(
                        "AllReduce", mybir.AluOpType.add,
                        replica_groups=_pairs_for_d(h),
                        ins=[cc_in.opt()], outs=[cc_out.opt()])
                    nc.sync.dma_start(t2[:], cc_out[:])
                    nc.vector.tensor_sub(S[:], t2[:], srcT[:])
                    srcT = S

            cur, nxt = AB, AB2
            for gi, g in enumerate(gates):
                Rt = load_R(gi)
                if g['d']:
                    fetch_partner(cur, g['d'])
                    src = S
                else:
                    src = cur
                nc.vector.tensor_mul(t[:, 0:2048], src[:, 0:2048], Rt[:, 0:2048])
                nc.vector.tensor_mul(t[:, 2048:4096], src[:, 2048:4096],
                                     Rt[:, 2048:4096])
                fhat = (g['chi'] << 11) | g['mf']
                calls = window_calls(fhat)
                perm = M[:, (2 * gi + 1) * P:(2 * gi + 2) * P]
                for h in range(2):
                    psh = (ps0, ps1)[h]
                    hsl = slice(h * 2048, (h + 1) * 2048)
                    # c*psi on ScalarE (overlaps the perm matmuls below)
                    nc.scalar.mul(nxt[:, hsl], cur[:, hsl], Cth[:, gi:gi + 1])
                    # each output column is written by exactly one xor-window
                    # call, so every call is its own accumulation group
                    for w in range(4):
                        wlo = h * 2048 + w * 512
                        wcalls = [cl for cl in calls if wlo <= cl[0] < wlo + 512]
                        for (out_off, in_off, out_dims, in_dims, cnt) in wcalls:
                            srcap = ap_with(t[:], in_off, in_dims)
                            dst = ap_with(psh[:], out_off - h * 2048, out_dims)
                            nc.tensor.matmul(dst, perm, srcap, start=True,
                                             stop=True)
                    nc.vector.tensor_add(nxt[:, hsl], nxt[:, hsl], psh[:])
                cur, nxt = nxt, cur

            # measurements, grouped by cross-core offset d
            for d in meas_d_order:
                if d:
                    fetch_partner(cur, d)
                    src = S
                else:
                    src = cur
                for mi, g in enumerate(meas):
                    if g['d'] != d:
                        continue
                    Rt = load_R(n_g + mi)
                    nc.vector.tensor_mul(t[:], src[:], Rt[:])
                    fhat = (g['chi'] << 11) | g['mf']
                    calls = window_calls(fhat)
                    perm = M[:, (2 * n_g + mi) * P:(2 * n_g + mi + 1) * P]
                    for h in range(2):
                        psh = (ps0, ps1)[h]
                        wcalls = [cl for cl in calls if h * 2048 <= cl[0] < (h + 1) * 2048]
                        for ci, (out_off, in_off, out_dims, in_dims, cnt) in enumerate(wcalls):
                            srcap = ap_with(t[:], in_off, in_dims)
                            dst = ap_with(psh[:], out_off - h * 2048, out_dims)
                            nc.tensor.matmul(dst, perm, srcap, start=True, stop=True)
                    nc.scalar.copy(t2[:, 0:2048], ps0[:])
                    nc.scalar.copy(t2[:, 2048:4096], ps1[:])
                    nc.gpsimd.tensor_mul(t2[:], cur[:], t2[:])
                    nc.vector.reduce_sum(accs[:, mi:mi + 1], t2[:],
                                         axis=mybir.AxisListType.X)
            # reduce partials across all cores so any single output shard is
            # complete: the host then fetches one 4 KB shard instead of eight
            acc_cc_in = dram.tile([P, n_m], DT, tag="acc_cc_in")
            acc_cc_out = dram.tile([P, n_m], DT, tag="acc_cc_out")
            nc.gpsimd.dma_start(acc_cc_in[:], accs[:])
            nc.gpsimd.collective_compute(
                "AllReduce", mybir.AluOpType.add,
                replica_groups=[list(range(N_CORES))],
                ins=[acc_cc_in.opt()], outs=[acc_cc_out.opt()])
            nc.sync.dma_start(accs[:], acc_cc_out[:])
            nc.sync.dma_start(acc_out[:, :], accs[:])
    nc.compile()
    return nc

# --------------------------------------------------------------- hw runner
def _make_sharded(nc, n_cores):
    import jax
    import numpy as _np
    import concourse.mybir as mybir
    from concourse.bass2jax import (_bass_exec_p, partition_id_tensor,
                                    install_neuronx_cc_hook)
    from jax.sharding import Mesh, PartitionSpec, NamedSharding
    from jax.experimental.shard_map import shard_map
    install_neuronx_cc_hook()
    partition_name = nc.partition_id_tensor.name if nc.partition_id_tensor else None
    in_names, out_names, out_avals, zero_outs = [], [], [], []
    for alloc in nc.m.functions[0].allocations:
        if not isinstance(alloc, mybir.MemoryLocationSet):
            continue
        name = alloc.memorylocations[0].name
        if alloc.kind == "ExternalInput":
            if name != partition_name:
                in_names.append(name)
        elif alloc.kind == "ExternalOutput":
            shape = tuple(alloc.tensor_shape)
            dtype = mybir.dt.np(alloc.dtype)
            out_avals.append(jax.core.ShapedArray(shape, dtype))
            out_names.append(name)
            zero_outs.append(_np.zeros(shape, dtype))
    n_params = len(in_names)
    all_in_names = in_names + out_names + ([partition_name] if partition_name else [])

    def _body(*args):
        operands = list(args)
        if partition_name is not None:
            operands.append(partition_id_tensor())
        outs = _bass_exec_p.bind(
            *operands, out_avals=tuple(out_avals), in_names=tuple(all_in_names),
            out_names=tuple(out_names), lowering_input_output_aliases=(),
            sim_require_finite=True, sim_require_nnan=True, nc=nc)
        return tuple(outs)

    devices = jax.devices()[:n_cores]
    mesh = Mesh(_np.asarray(devices), ("core",))
    sharding = NamedSharding(mesh, PartitionSpec("core"))
    sharded = jax.jit(
        shard_map(_body, mesh=mesh,
                  in_specs=(PartitionSpec("core"),) * (n_params + len(out_names)),
                  out_specs=(PartitionSpec("core"),) * len(out_names), check_rep=False),
        keep_unused=True)
    return dict(sharded=sharded, in_names=in_names, out_names=out_names,
                out_avals=out_avals, zero_outs=zero_outs, sharding=sharding)

def _device_put(arr, sharding):
    import jax
    d = jax.device_put(arr, sharding)
    d.block_until_ready()
    return d

# ------------------------------------------------------------------ kernel
_PLAN_CACHE = {}
_TAB_CACHE = {}
_FEAT_CACHE = {}

def kernel(feature, theta, gate_flip, gate_pmask, gate_ny,
           meas_flip, meas_pmask, meas_ny):
    feature = np.asarray(feature)
    theta64 = np.asarray(theta, np.float64)
    gf = [int(x) for x in np.asarray(gate_flip)]
    gp = [int(x) for x in np.asarray(gate_pmask)]
    gn = [int(x) for x in np.asarray(gate_ny)]
    mf = [int(x) for x in np.asarray(meas_flip)]
    mp = [int(x) for x in np.asarray(meas_pmask)]
    mn = [int(x) for x in np.asarray(meas_ny)]

    key = (tuple(gf), tuple(gp), tuple(gn), tuple(mf), tuple(mp), tuple(mn))
    plan = _PLAN_CACHE.get(key)
    if plan is None:
        ph = _plan_sharding(gf, mf)
        gates = []
        for i in range(N_GATES):
            g = gate_local(ph, gf[i], gp[i], gn[i], strict=False)
            g['chi'] = gate_coeffs(gn[i], 0, 0)[0]
            g['d'] = ph.core_of(gf[i])
            gates.append(g)
        meas = []
        for m in range(N_MEAS):
            g = gate_local(ph, mf[m], mp[m], mn[m], strict=False)
            g['chi'] = meas_coeffs(mn[m])[0]
            g['d'] = ph.core_of(mf[m])
            meas.append(g)
        ds = sorted({g['d'] for g in meas})
        meas_d_order = ([0] if 0 in ds else []) + [d for d in ds if d]
        nc = _build_nc(gates, meas, meas_d_order)
        runner = _make_sharded(nc, N_CORES)
        l = np.arange(1 << NLOC, dtype=np.int64)
        jmap = [ph.global_of_vec(np.full_like(l, c), l.copy()) for c in range(8)]
        plan = dict(phase=ph, gates=gates, meas=meas, runner=runner, jmap=jmap,
                    zeros_dev=None)
        _PLAN_CACHE[key] = plan

    runner = plan['runner']
    sharding = runner['sharding']
    gates, meas = plan['gates'], plan['meas']

    # theta-dependent tables -> device (cached)
    tkey = (key, theta64.tobytes())
    tab = _TAB_CACHE.get(tkey)
    if tab is None:
        cth = np.cos(theta64[:, 0] / 2)
        sth = np.sin(theta64[:, 0] / 2)
        rr_all, mats_all = [], []
        for c in range(8):
            rows = [build_R(g, c, *gate_coeffs(gn[i], cth[i], sth[i])[1:])
                    for i, g in enumerate(gates)]
            rows += [build_R(g, c, *meas_coeffs(mn[m])[1:])
                     for m, g in enumerate(meas)]
            rr_all.append(np.stack(rows))
            ms = []
            for i, g in enumerate(gates):
                dg, pm_ = build_mats(g, cth[i], c)
                ms.append(dg); ms.append(pm_)
            for m, g in enumerate(meas):
                ms.append(build_mats(g, 1.0, c)[1])
            mats_all.append(np.stack(ms))
        rr_dev = _device_put(np.concatenate(rr_all, axis=0), sharding)
        mats_dev = _device_put(np.concatenate(mats_all, axis=0), sharding)
        cth_row = np.tile(cth.astype(np.float32)[None, :], (P, 1))
        cth_dev = _device_put(np.concatenate([cth_row] * 8, axis=0), sharding)
        tab = (rr_dev, mats_dev, cth_dev)
        _TAB_CACHE[tkey] = tab
    rr_dev, mats_dev, cth_dev = tab

    if plan['zeros_dev'] is None:
        plan['zeros_dev'] = [
            _device_put(np.zeros((N_CORES * z.shape[0],) + z.shape[1:], z.dtype),
                        sharding) for z in runner['zero_outs']]

    def dispatch(ab_dev):
        in_map = {"ab_in": ab_dev, "r_rows": rr_dev, "mats": mats_dev,
                  "cth": cth_dev}
        args = [in_map[n] for n in runner['in_names']] + plan['zeros_dev']
        comp = plan.get('compiled')
        if comp is None:
            comp = runner['sharded'].lower(*args).compile()
            plan['compiled'] = comp
            plan['signs'] = np.array([1.0 if (x % 4) in (0, 1) else -1.0
                                      for x in mn])
        return comp(*args)

    # feature-dependent statevector shards -> device (cached by content).
    # With a cache hit candidate, launch first and verify the 8 MB content
    # while the dispatch round trip is in flight; a mismatch discards the
    # speculative result before it is ever used.
    fent = _FEAT_CACHE.get(key)
    out_arrs = None
    if (fent is not None and plan.get('compiled') is not None and
            fent[0].dtype == feature.dtype and fent[0].shape == feature.shape):
        out_arrs = dispatch(fent[1])
        if not np.array_equal(fent[0], feature):
            out_arrs = None
            fent = None
    elif fent is not None and not (fent[0].dtype == feature.dtype and
                                   np.array_equal(fent[0], feature)):
        fent = None
    if fent is None:
        f64 = feature.astype(np.float64)
        nrm2 = float((f64 ** 2).sum())
        f32 = feature.astype(np.float32)
        abs_ = []
        for c in range(8):
            a = f32[plan['jmap'][c]].reshape(P, NF)
            abs_.append(np.concatenate([a, np.zeros_like(a)], axis=1))
        ab_dev = _device_put(np.concatenate(abs_, axis=0), sharding)
        fent = (feature.copy(), ab_dev, nrm2)
        _FEAT_CACHE[key] = fent
    nrm2 = fent[2]
    if out_arrs is None:
        out_arrs = dispatch(fent[1])
    # every shard holds the core-reduced totals; fetch just one
    accs = np.asarray(out_arrs[0].addressable_shards[0].data).astype(np.float64)
    return plan['signs'] * accs.sum(axis=0) / nrm2
